# revision 17
# baseline (speedup 1.0000x reference)
"""Trainium2 Bass kernel for nn_ArchetipesNetwork (multi-module RON recurrence).

Distribution: module-parallel over 8 NeuronCores (2 of 16 modules per core).
The feedback einsum  fb[i,h1] = scaling[i] * sum_{o,h2} conn[i,o]*wm[i,o,h1,h2]*outs[o,h2]
is a fixed 4096x4096 matvec against the concatenated hidden state hy.  conn and
scaling are folded into wm on the host; each core keeps its 512 rows of the
folded matrix resident in SBUF (bf16) for all T steps.  Per step:

  TensorE : psum_fb   += A_chunk^T @ hy_chunk   (32 chunks of 128, 4 out-tiles)
            psum_rest  = W_in@x_t + bias + W_rec@hy_own
            PE-transpose of the bf16 hy shard [128,4] -> [4,128] so the bounce
            write is 4x256B page-aligned HBM chunks (8B-per-partition writes
            cost ~6us to commit; this form commits in ~1us)
  VectorE : fb copy-out, pre-add, state update (scaled state z = DT*hz),
            bf16 cast of hy, PSUM->SBUF copy of the transposed shard
  ScalarE : tanh, z*C1 prescale (off critical path)
  gpsimd  : per-step AllGather trigger (ncfw mesh) over DRAM bounce buffers
  SyncE   : initial weight loads, bounce out/in DMAs, final output DMAs

Everything is fully unrolled over T with hand-placed semaphores; dependent
back-to-back DVE ops carry explicit drain()s (deep-pipeline RAW hazard).
Measured: 4.80 ms HW exec (18.7 us/step), rel_err 3.7e-3 vs reference.
Per-step critical chain: mesh AllGather ~6.4us + doorbell/dma legs ~5us +
matvec block 3.6us (LDWEIGHTS-bound, 27ns per 128x128 bf16 chunk) + ~3us
elementwise/transpose.
"""

import numpy as np
import ml_dtypes

import concourse.bass as bass
import concourse.mybir as mybir
from concourse.bass_utils import run_bass_kernel_spmd

F32 = mybir.dt.float32
BF16 = mybir.dt.bfloat16
BF16_NP = ml_dtypes.bfloat16

M, H, I = 16, 256, 128
DT_C = 0.042
GAMMA = 2.7
EPS = 4.7
NCORES = 8
C1 = 1.0 - DT_C * EPS       # z' = C1*z + DT^2*(tanh(pre) - GAMMA*hy)
C3 = DT_C * DT_C

# hy_all chunk mapping: flat f = p*32 + c over agout (concat of per-core
# transposed shards agin [4,128], i.e. flat = s*512 + q*128 + p'):
#   s = f>>9, q = (f&511)>>7, p' = f&127 ; element = hy[2*s + (q>>1), (q&1)*128 + p']
_P = np.arange(128)[:, None]
_C = np.arange(32)[None, :]
_F = _P * 32 + _C
_S = _F >> 9
_Q = (_F & 511) >> 7
_PP = _F & 127
_M_MAT = 2 * _S + (_Q >> 1)           # [128,32] module index of element (p,c)
_H_MAT = (_Q & 1) * 128 + _PP         # [128,32] h index of element (p,c)


def build_nc(T):
    nc = bass.Bass(target_bir_lowering=False, debug=False, num_devices=NCORES)

    a_ext = nc.declare_dram_parameter("a_pack", [128, 4 * 32 * 128], BF16, isOutput=False)
    wrec_ext = nc.declare_dram_parameter("wrec_pack", [128, 4 * 2 * 128], BF16, isOutput=False)
    win_ext = nc.declare_dram_parameter("win_pack", [128, 4 * 128], BF16, isOutput=False)
    bias_ext = nc.declare_dram_parameter("bias_pack", [1, 512], BF16, isOutput=False)
    xt_ext = nc.declare_dram_parameter("xt", [128, T], BF16, isOutput=False)
    mask_ext = nc.declare_dram_parameter("mask_rep", [128, 2], F32, isOutput=False)
    id_ext = nc.declare_dram_parameter("id_pack", [128, 128], BF16, isOutput=False)
    out_hy = nc.declare_dram_parameter("out_hy", [128, 4 * T], F32, isOutput=True)
    out_hz = nc.declare_dram_parameter("out_hz", [128, 4 * T], F32, isOutput=True)
    out_fb = nc.declare_dram_parameter("out_fb", [128, 4 * T], F32, isOutput=True)

    # collective bounce buffers (double-buffered)
    agin = [nc.dram_tensor(f"agin{k}", [4, 128], BF16) for k in range(2)]
    agout = [nc.dram_tensor(f"agout{k}", [128, 32], BF16) for k in range(2)]

    rg = [list(range(NCORES))]

    from contextlib import ExitStack
    with ExitStack() as ctx:
        block = ctx.enter_context(nc.Block())
        a_sb = ctx.enter_context(nc.sbuf_tensor("a_sb", [128, 4 * 32 * 128], BF16))
        wrec_sb = ctx.enter_context(nc.sbuf_tensor("wrec_sb", [128, 4 * 2 * 128], BF16))
        win_sb = ctx.enter_context(nc.sbuf_tensor("win_sb", [128, 4 * 128], BF16))
        bias_sb = ctx.enter_context(nc.sbuf_tensor("bias_sb", [1, 512], BF16))
        one_sb = ctx.enter_context(nc.sbuf_tensor("one_sb", [1, 1], BF16))
        xt_sb = ctx.enter_context(nc.sbuf_tensor("xt_sb", [128, T], BF16))
        mask_sb = ctx.enter_context(nc.sbuf_tensor("mask_sb", [128, 2], F32))
        hy16_sb = ctx.enter_context(nc.sbuf_tensor("hy16_sb", [128, 8], BF16))
        hyall_sb = ctx.enter_context(nc.sbuf_tensor("hyall_sb", [128, 64], BF16))
        obuf_hy = ctx.enter_context(nc.sbuf_tensor("obuf_hy", [128, 4 * (T + 1)], F32))
        obuf_hz = ctx.enter_context(nc.sbuf_tensor("obuf_hz", [128, 4 * (T + 1)], F32))
        obuf_fb = ctx.enter_context(nc.sbuf_tensor("obuf_fb", [128, 4 * T], F32))
        pre_sb = ctx.enter_context(nc.sbuf_tensor("pre_sb", [128, 8], F32))
        w_sb = ctx.enter_context(nc.sbuf_tensor("w_sb", [128, 8], F32))
        tnh_sb = ctx.enter_context(nc.sbuf_tensor("tnh_sb", [128, 8], F32))
        zc1_sb = ctx.enter_context(nc.sbuf_tensor("zc1_sb", [128, 8], F32))
        rb_sb = ctx.enter_context(nc.sbuf_tensor("rb_sb", [4, 128], BF16))
        id_sb = ctx.enter_context(nc.sbuf_tensor("id_sb", [128, 128], BF16))
        tr_sb = ctx.enter_context(nc.sbuf_tensor("tr_sb", [4, 256], BF16))
        psum_fb0 = ctx.enter_context(nc.psum_tensor([128, 4], F32))
        psum_fb1 = ctx.enter_context(nc.psum_tensor([128, 4], F32))
        psum_rest0 = ctx.enter_context(nc.psum_tensor([128, 4], F32))
        psum_rest1 = ctx.enter_context(nc.psum_tensor([128, 4], F32))
        psum_tr0 = ctx.enter_context(nc.psum_tensor([4, 128], BF16))
        psum_tr1 = ctx.enter_context(nc.psum_tensor([4, 128], BF16))
        ldsem = ctx.enter_context(nc.semaphore("ldsem"))
        isem = ctx.enter_context(nc.semaphore("isem"))
        mmsem = ctx.enter_context(nc.semaphore("mmsem"))
        predone = ctx.enter_context(nc.semaphore("predone"))
        fbdone = ctx.enter_context(nc.semaphore("fbdone"))
        tanhsem = ctx.enter_context(nc.semaphore("tanhsem"))
        zc1sem = ctx.enter_context(nc.semaphore("zc1sem"))
        zdone = ctx.enter_context(nc.semaphore("zdone"))
        ssem = ctx.enter_context(nc.semaphore("ssem"))
        d1sem = ctx.enter_context(nc.semaphore("d1sem"))
        rbsem = ctx.enter_context(nc.semaphore("rbsem"))
        castsem = ctx.enter_context(nc.semaphore("castsem"))
        trsem = ctx.enter_context(nc.semaphore("trsem"))
        ccsem = ctx.enter_context(nc.semaphore("ccsem"))
        d2sem = ctx.enter_context(nc.semaphore("d2sem"))
        fsem = ctx.enter_context(nc.semaphore("fsem"))
        osem = ctx.enter_context(nc.semaphore("osem"))
        psum_fb = [psum_fb0, psum_fb1]
        psum_rest = [psum_rest0, psum_rest1]
        psum_tr = [psum_tr0, psum_tr1]

        @block.sync
        def _(sync):
            sync.dma_start(out=a_sb[:, :], in_=a_ext[:, :]).then_inc(ldsem, 16)
            sync.dma_start(out=wrec_sb[:, :], in_=wrec_ext[:, :]).then_inc(ldsem, 16)
            sync.dma_start(out=win_sb[:, :], in_=win_ext[:, :]).then_inc(ldsem, 16)
            sync.dma_start(out=bias_sb[:, :], in_=bias_ext[:, :]).then_inc(ldsem, 16)
            sync.dma_start(out=xt_sb[:, :], in_=xt_ext[:, :]).then_inc(ldsem, 16)
            sync.dma_start(out=mask_sb[:, :], in_=mask_ext[:, :]).then_inc(ldsem, 16)
            sync.dma_start(out=id_sb[:, :], in_=id_ext[:, :]).then_inc(ldsem, 16)
            for s in range(T - 1):
                p = s % 2
                sync.wait_ge(ssem, s + 1)
                sync.dma_start(out=agin[p][:, :], in_=tr_sb[0:4, 128 * p : 128 * p + 128]).then_inc(d1sem, 16)
                sync.wait_ge(ccsem, s + 1)
                sync.dma_start(out=hyall_sb[:, 32 * p : 32 * p + 32], in_=agout[p][:, :]).then_inc(d2sem, 16)
            sync.wait_ge(fsem, 1)
            sync.dma_start(out=out_hy[:, :], in_=obuf_hy[:, 4 : 4 * (T + 1)]).then_inc(osem, 16)
            sync.dma_start(out=out_hz[:, :], in_=obuf_hz[:, 4 : 4 * (T + 1)]).then_inc(osem, 16)
            sync.dma_start(out=out_fb[:, :], in_=obuf_fb[:, :]).then_inc(osem, 16)
            sync.wait_ge(osem, 48)

        @block.gpsimd
        def _(gpsimd):
            gpsimd.memset(one_sb[:, :], 1.0).then_inc(isem, 1)
            gpsimd.memset(hy16_sb[:, :], 0.0).then_inc(isem, 1)
            gpsimd.memset(hyall_sb[:, 32:64], 0.0).then_inc(isem, 1)
            gpsimd.memset(obuf_hy[:, 0:4], 0.0).then_inc(isem, 1)
            gpsimd.memset(obuf_hz[:, 0:4], 0.0).then_inc(isem, 1)
            for s in range(T - 1):
                p = s % 2
                gpsimd.wait_ge(d1sem, 16 * (s + 1))
                if s >= 2:
                    gpsimd.wait_ge(d2sem, 16 * (s - 1))
                gpsimd.collective_compute(
                    "AllGather",
                    mybir.AluOpType.bypass,
                    replica_groups=rg,
                    ins=[agin[p].ap().opt()],
                    outs=[agout[p].ap().opt()],
                ).then_inc(ccsem, 1)

        @block.tensor
        def _(tensor):
            tensor.wait_ge(ldsem, 112)
            tensor.wait_ge(isem, 7)
            for t in range(T):
                p = t % 2
                pm = (t - 1) % 2
                if t >= 2:
                    tensor.wait_ge(predone, t - 1)
                if t >= 1:
                    tensor.wait_ge(ssem, t)
                # psum_rest: W_in@x_t + bias + W_rec@hy_own
                for j in range(4):
                    li = j // 2
                    outp = psum_rest[p][:, j : j + 1]
                    tensor.matmul(
                        outp,
                        win_sb[:, j * 128 : (j + 1) * 128],
                        xt_sb[:, t : t + 1],
                        start=True, stop=False,
                    )
                    tensor.matmul(
                        outp,
                        bias_sb[0:1, j * 128 : (j + 1) * 128],
                        one_sb[0:1, 0:1],
                        start=False, stop=False,
                    )
                    for r in range(2):
                        tensor.matmul(
                            outp,
                            wrec_sb[:, (j * 2 + r) * 128 : (j * 2 + r + 1) * 128],
                            hy16_sb[:, 4 * pm + 2 * li + r : 4 * pm + 2 * li + r + 1],
                            start=False, stop=(r == 1),
                        )
                # psum_fb: folded feedback matvec over gathered hy
                if t >= 1:
                    tensor.wait_ge(d2sem, 16 * t)
                for j in range(4):
                    for cc in range(32):
                        mm = tensor.matmul(
                            psum_fb[p][:, j : j + 1],
                            a_sb[:, (j * 32 + cc) * 128 : (j * 32 + cc + 1) * 128],
                            hyall_sb[:, 32 * pm + cc : 32 * pm + cc + 1],
                            start=(cc == 0), stop=(cc == 31),
                        )
                mm.then_inc(mmsem, 1)
                if t <= T - 2:
                    tensor.wait_ge(castsem, t + 1)
                    tensor.transpose(
                        psum_tr[p][0:4, :],
                        hy16_sb[:, 4 * p : 4 * p + 4],
                        id_sb[:, :],
                    ).then_inc(trsem, 1)

        @block.vector
        def _(vector):
            vector.wait_ge(ldsem, 112)
            for li in range(2):
                vector.tensor_scalar(
                    win_sb[:, li * 256 : (li + 1) * 256],
                    win_sb[:, li * 256 : (li + 1) * 256],
                    mask_sb[:, li : li + 1],
                    None,
                    mybir.AluOpType.mult,
                ).then_inc(isem, 1)
            for t in range(T):
                p = t % 2
                vector.wait_ge(mmsem, t + 1)
                vector.tensor_copy(
                    obuf_fb[:, 4 * t : 4 * t + 4],
                    psum_fb[p][:, :],
                )
                vector.drain()
                vector.tensor_tensor(
                    pre_sb[:, 4 * p : 4 * p + 4],
                    psum_rest[p][:, :],
                    obuf_fb[:, 4 * t : 4 * t + 4],
                    mybir.AluOpType.add,
                ).then_inc(predone, 1)
                vector.wait_ge(tanhsem, t + 1)
                vector.scalar_tensor_tensor(
                    w_sb[:, 4 * p : 4 * p + 4],
                    obuf_hy[:, 4 * t : 4 * t + 4],
                    -GAMMA,
                    tnh_sb[:, 4 * p : 4 * p + 4],
                    mybir.AluOpType.mult,
                    mybir.AluOpType.add,
                )
                vector.wait_ge(zc1sem, t + 1)
                vector.drain()
                vector.scalar_tensor_tensor(
                    obuf_hz[:, 4 * (t + 1) : 4 * (t + 2)],
                    w_sb[:, 4 * p : 4 * p + 4],
                    C3,
                    zc1_sb[:, 4 * p : 4 * p + 4],
                    mybir.AluOpType.mult,
                    mybir.AluOpType.add,
                ).then_inc(zdone, 1)
                vector.drain()
                vector.tensor_tensor(
                    hy16_sb[:, 4 * p : 4 * p + 4],
                    obuf_hy[:, 4 * t : 4 * t + 4],
                    obuf_hz[:, 4 * (t + 1) : 4 * (t + 2)],
                    mybir.AluOpType.add,
                ).then_inc(castsem, 1)
                vector.tensor_tensor(
                    obuf_hy[:, 4 * (t + 1) : 4 * (t + 2)],
                    obuf_hy[:, 4 * t : 4 * t + 4],
                    obuf_hz[:, 4 * (t + 1) : 4 * (t + 2)],
                    mybir.AluOpType.add,
                )
                if t <= T - 2:
                    vector.wait_ge(trsem, t + 1)
                    vector.tensor_copy(
                        tr_sb[0:4, 128 * p : 128 * p + 128],
                        psum_tr[p][0:4, :],
                    ).then_inc(ssem, 1)
                vector.drain()
            # hz output = z / DT
            vector.drain()
            vector.tensor_scalar_mul(
                obuf_hz[:, 4 : 4 * (T + 1)],
                obuf_hz[:, 4 : 4 * (T + 1)],
                1.0 / DT_C,
            ).then_inc(fsem, 1)

        @block.scalar
        def _(scalar):
            scalar.wait_ge(isem, 7)
            for t in range(T):
                p = t % 2
                if t >= 1:
                    scalar.wait_ge(zdone, t)
                scalar.activation(
                    zc1_sb[:, 4 * p : 4 * p + 4],
                    obuf_hz[:, 4 * t : 4 * t + 4],
                    mybir.ActivationFunctionType.Copy,
                    scale=C1,
                ).then_inc(zc1sem, 1)
                scalar.wait_ge(predone, t + 1)
                scalar.activation(
                    tnh_sb[:, 4 * p : 4 * p + 4],
                    pre_sb[:, 4 * p : 4 * p + 4],
                    mybir.ActivationFunctionType.Tanh,
                ).then_inc(tanhsem, 1)

    return nc


def pack_inputs(x, wm, conn, mask, W_in, W_rec, bias):
    """Build the 8 per-core input dicts (host-side layout marshaling)."""
    T = x.shape[0]
    scaling = 1.0 / np.clip(conn.sum(axis=1), 1.0, None)
    abar = (conn[:, :, None, None] * scaling[:, None, None, None] * wm).astype(np.float32)

    xt = np.ascontiguousarray(x.T).astype(BF16_NP)  # [128, T]

    in_maps = []
    for c in range(NCORES):
        a_pack = np.zeros((128, 4 * 32 * 128), np.float32)
        wrec_pack = np.zeros((128, 4 * 2 * 128), np.float32)
        win_pack = np.zeros((128, 4 * 128), np.float32)
        bias_pack = np.zeros((1, 512), np.float32)
        for j in range(4):
            li, h1f = j // 2, j % 2
            i = 2 * c + li
            # A blocks: [p, c32, h1] -> cols (j*32+c32)*128 + mcol
            vals = abar[i][_M_MAT, :, _H_MAT]            # [128, 32, 256]
            vals = vals[:, :, h1f * 128 : (h1f + 1) * 128]  # [128, 32, 128]
            a_pack[:, j * 4096 : (j + 1) * 4096] = vals.reshape(128, 4096)
            wrt = W_rec[i].T                              # [h2, h1]
            for r in range(2):
                wrec_pack[:, (j * 2 + r) * 128 : (j * 2 + r + 1) * 128] = \
                    wrt[r * 128 : (r + 1) * 128, h1f * 128 : (h1f + 1) * 128]
            win_pack[:, j * 128 : (j + 1) * 128] = \
                W_in[i].T[:, h1f * 128 : (h1f + 1) * 128]
            bias_pack[0, j * 128 : (j + 1) * 128] = bias[i, h1f * 128 : (h1f + 1) * 128]
        mask_rep = np.broadcast_to(mask[2 * c : 2 * c + 2][None, :], (128, 2))
        in_maps.append({
            "id_pack": np.eye(128, dtype=BF16_NP),
            "a_pack": a_pack.astype(BF16_NP),
            "wrec_pack": wrec_pack.astype(BF16_NP),
            "win_pack": win_pack.astype(BF16_NP),
            "bias_pack": bias_pack.astype(BF16_NP),
            "xt": xt,
            "mask_rep": np.ascontiguousarray(mask_rep).astype(np.float32),
        })
    return in_maps


def unpack_outputs(results, T):
    states = np.zeros((T, M, 2, H), np.float32)
    fb = np.zeros((T, M, H), np.float32)
    for c in range(NCORES):
        o_hy = results[c]["out_hy"]  # [128, 4T]
        o_hz = results[c]["out_hz"]
        o_fb = results[c]["out_fb"]
        for j in range(4):
            li, h1f = j // 2, j % 2
            m = 2 * c + li
            hs = slice(h1f * 128, (h1f + 1) * 128)
            states[:, m, 0, hs] = o_hy[:, j::4].T
            states[:, m, 1, hs] = o_hz[:, j::4].T
            fb[:, m, hs] = o_fb[:, j::4].T
    return states, fb


_NC_CACHE = {}


def run_packed(in_maps, T, trace=False):
    if T not in _NC_CACHE:
        _NC_CACHE[T] = build_nc(T)
    nc = _NC_CACHE[T]
    res = run_bass_kernel_spmd(nc, in_maps, core_ids=list(range(NCORES)), trace=trace)
    return res


def kernel(x, wm, connection_weights, input_mask, W_in, W_rec, bias):
    x = np.asarray(x, np.float32)
    wm = np.asarray(wm, np.float32)
    conn = np.asarray(connection_weights, np.float32)
    mask = np.asarray(input_mask, np.float32)
    W_in = np.asarray(W_in, np.float32)
    W_rec = np.asarray(W_rec, np.float32)
    bias = np.asarray(bias, np.float32)
    T = x.shape[0]
    in_maps = pack_inputs(x, wm, conn, mask, W_in, W_rec, bias)
    res = run_packed(in_maps, T, trace=False)
    return unpack_outputs(res.results, T)


# revision 18
# speedup vs baseline: 1.0859x; 1.0859x over previous
"""Trainium2 Bass kernel for nn_ArchetipesNetwork (multi-module RON recurrence).

Distribution: module-parallel over 8 NeuronCores (2 of 16 modules per core).
The feedback einsum  fb[i,h1] = scaling[i] * sum_{o,h2} conn[i,o]*wm[i,o,h1,h2]*outs[o,h2]
is a fixed 4096x4096 matvec against the concatenated hidden state hy.  conn and
scaling are folded into wm on the host; each core keeps its 512 rows of the
folded matrix resident in SBUF (bf16) for all T steps.  Per step:

  TensorE : psum_fb   += A_chunk^T @ hy_chunk   (32 chunks of 128, 4 out-tiles)
            psum_rest  = W_in@x_t + bias + W_rec@hy_own
            PE-transpose of the bf16 hy shard [128,4] -> [4,128] so the bounce
            write is 4x256B page-aligned HBM chunks (8B-per-partition writes
            cost ~6us to commit; this form commits in ~1us)
  VectorE : fb copy-out, pre-add, state update (scaled state z = DT*hz),
            bf16 cast of hy, PSUM->SBUF copy of the transposed shard
  ScalarE : tanh, z*C1 prescale (off critical path)
  gpsimd  : per-step AllGather trigger (ncfw mesh) over DRAM bounce buffers
  SyncE   : initial weight loads, bounce out/in DMAs, final output DMAs

Everything is fully unrolled over T with hand-placed semaphores; dependent
back-to-back DVE ops carry explicit drain()s (deep-pipeline RAW hazard).
Measured: 4.80-5.18 ms HW exec (18.7-20.2 us/step, run-to-run machine
noise ~7%), rel_err 3.7e-3 vs reference.
Per-step critical chain: mesh AllGather ~6.4us + doorbell/dma legs ~5us +
matvec block 3.6us (LDWEIGHTS-bound, 27ns per 128x128 bf16 chunk) + ~3us
elementwise/transpose.
"""

import numpy as np
import ml_dtypes

import concourse.bass as bass
import concourse.mybir as mybir
from concourse.bass_utils import run_bass_kernel_spmd

F32 = mybir.dt.float32
BF16 = mybir.dt.bfloat16
BF16_NP = ml_dtypes.bfloat16

M, H, I = 16, 256, 128
DT_C = 0.042
GAMMA = 2.7
EPS = 4.7
NCORES = 8
C1 = 1.0 - DT_C * EPS       # z' = C1*z + DT^2*(tanh(pre) - GAMMA*hy)
C3 = DT_C * DT_C

# hy_all chunk mapping: flat f = p*32 + c over agout (concat of per-core
# transposed shards agin [4,128], i.e. flat = s*512 + q*128 + p'):
#   s = f>>9, q = (f&511)>>7, p' = f&127 ; element = hy[2*s + (q>>1), (q&1)*128 + p']
_P = np.arange(128)[:, None]
_C = np.arange(32)[None, :]
_F = _P * 32 + _C
_S = _F >> 9
_Q = (_F & 511) >> 7
_PP = _F & 127
_M_MAT = 2 * _S + (_Q >> 1)           # [128,32] module index of element (p,c)
_H_MAT = (_Q & 1) * 128 + _PP         # [128,32] h index of element (p,c)


def build_nc(T):
    nc = bass.Bass(target_bir_lowering=False, debug=False, num_devices=NCORES)

    a_ext = nc.declare_dram_parameter("a_pack", [128, 4 * 32 * 128], BF16, isOutput=False)
    wrec_ext = nc.declare_dram_parameter("wrec_pack", [128, 4 * 2 * 128], BF16, isOutput=False)
    win_ext = nc.declare_dram_parameter("win_pack", [128, 4 * 128], BF16, isOutput=False)
    bias_ext = nc.declare_dram_parameter("bias_pack", [1, 512], BF16, isOutput=False)
    xt_ext = nc.declare_dram_parameter("xt", [128, T], BF16, isOutput=False)
    mask_ext = nc.declare_dram_parameter("mask_rep", [128, 2], F32, isOutput=False)
    id_ext = nc.declare_dram_parameter("id_pack", [128, 128], BF16, isOutput=False)
    out_hy = nc.declare_dram_parameter("out_hy", [128, 4 * T], F32, isOutput=True)
    out_hz = nc.declare_dram_parameter("out_hz", [128, 4 * T], F32, isOutput=True)
    out_fb = nc.declare_dram_parameter("out_fb", [128, 4 * T], F32, isOutput=True)

    # collective bounce buffers (double-buffered)
    agin = [nc.dram_tensor(f"agin{k}", [4, 128], BF16) for k in range(2)]
    agout = [nc.dram_tensor(f"agout{k}", [128, 32], BF16) for k in range(2)]

    rg = [list(range(NCORES))]

    from contextlib import ExitStack
    with ExitStack() as ctx:
        block = ctx.enter_context(nc.Block())
        a_sb = ctx.enter_context(nc.sbuf_tensor("a_sb", [128, 4 * 32 * 128], BF16))
        wrec_sb = ctx.enter_context(nc.sbuf_tensor("wrec_sb", [128, 4 * 2 * 128], BF16))
        win_sb = ctx.enter_context(nc.sbuf_tensor("win_sb", [128, 4 * 128], BF16))
        bias_sb = ctx.enter_context(nc.sbuf_tensor("bias_sb", [1, 512], BF16))
        one_sb = ctx.enter_context(nc.sbuf_tensor("one_sb", [1, 1], BF16))
        xt_sb = ctx.enter_context(nc.sbuf_tensor("xt_sb", [128, T], BF16))
        mask_sb = ctx.enter_context(nc.sbuf_tensor("mask_sb", [128, 2], F32))
        hy16_sb = ctx.enter_context(nc.sbuf_tensor("hy16_sb", [128, 8], BF16))
        hyall_sb = ctx.enter_context(nc.sbuf_tensor("hyall_sb", [128, 64], BF16))
        obuf_hy = ctx.enter_context(nc.sbuf_tensor("obuf_hy", [128, 4 * (T + 1)], F32))
        obuf_hz = ctx.enter_context(nc.sbuf_tensor("obuf_hz", [128, 4 * (T + 1)], F32))
        obuf_fb = ctx.enter_context(nc.sbuf_tensor("obuf_fb", [128, 4 * T], F32))
        pre_sb = ctx.enter_context(nc.sbuf_tensor("pre_sb", [128, 8], F32))
        w_sb = ctx.enter_context(nc.sbuf_tensor("w_sb", [128, 8], F32))
        tnh_sb = ctx.enter_context(nc.sbuf_tensor("tnh_sb", [128, 8], F32))
        zc1_sb = ctx.enter_context(nc.sbuf_tensor("zc1_sb", [128, 8], F32))
        rb_sb = ctx.enter_context(nc.sbuf_tensor("rb_sb", [4, 128], BF16))
        id_sb = ctx.enter_context(nc.sbuf_tensor("id_sb", [128, 128], BF16))
        tr_sb = ctx.enter_context(nc.sbuf_tensor("tr_sb", [4, 256], BF16))
        psum_fb0 = ctx.enter_context(nc.psum_tensor([128, 4], F32))
        psum_fb1 = ctx.enter_context(nc.psum_tensor([128, 4], F32))
        psum_rest0 = ctx.enter_context(nc.psum_tensor([128, 4], F32))
        psum_rest1 = ctx.enter_context(nc.psum_tensor([128, 4], F32))
        psum_tr0 = ctx.enter_context(nc.psum_tensor([4, 128], BF16))
        psum_tr1 = ctx.enter_context(nc.psum_tensor([4, 128], BF16))
        ldsem = ctx.enter_context(nc.semaphore("ldsem"))
        isem = ctx.enter_context(nc.semaphore("isem"))
        mmsem = ctx.enter_context(nc.semaphore("mmsem"))
        predone = ctx.enter_context(nc.semaphore("predone"))
        fbdone = ctx.enter_context(nc.semaphore("fbdone"))
        tanhsem = ctx.enter_context(nc.semaphore("tanhsem"))
        zc1sem = ctx.enter_context(nc.semaphore("zc1sem"))
        zdone = ctx.enter_context(nc.semaphore("zdone"))
        ssem = ctx.enter_context(nc.semaphore("ssem"))
        d1sem = ctx.enter_context(nc.semaphore("d1sem"))
        rbsem = ctx.enter_context(nc.semaphore("rbsem"))
        castsem = ctx.enter_context(nc.semaphore("castsem"))
        trsem = ctx.enter_context(nc.semaphore("trsem"))
        ccsem = ctx.enter_context(nc.semaphore("ccsem"))
        d2sem = ctx.enter_context(nc.semaphore("d2sem"))
        fsem = ctx.enter_context(nc.semaphore("fsem"))
        osem = ctx.enter_context(nc.semaphore("osem"))
        psum_fb = [psum_fb0, psum_fb1]
        psum_rest = [psum_rest0, psum_rest1]
        psum_tr = [psum_tr0, psum_tr1]

        @block.sync
        def _(sync):
            sync.dma_start(out=a_sb[:, :], in_=a_ext[:, :]).then_inc(ldsem, 16)
            sync.dma_start(out=wrec_sb[:, :], in_=wrec_ext[:, :]).then_inc(ldsem, 16)
            sync.dma_start(out=win_sb[:, :], in_=win_ext[:, :]).then_inc(ldsem, 16)
            sync.dma_start(out=bias_sb[:, :], in_=bias_ext[:, :]).then_inc(ldsem, 16)
            sync.dma_start(out=xt_sb[:, :], in_=xt_ext[:, :]).then_inc(ldsem, 16)
            sync.dma_start(out=mask_sb[:, :], in_=mask_ext[:, :]).then_inc(ldsem, 16)
            sync.dma_start(out=id_sb[:, :], in_=id_ext[:, :]).then_inc(ldsem, 16)
            for s in range(T - 1):
                p = s % 2
                sync.wait_ge(ssem, s + 1)
                sync.dma_start(out=agin[p][:, :], in_=tr_sb[0:4, 128 * p : 128 * p + 128]).then_inc(d1sem, 16)
                sync.wait_ge(ccsem, s + 1)
                sync.dma_start(out=hyall_sb[:, 32 * p : 32 * p + 32], in_=agout[p][:, :]).then_inc(d2sem, 16)
            sync.wait_ge(fsem, 1)
            sync.dma_start(out=out_hy[:, :], in_=obuf_hy[:, 4 : 4 * (T + 1)]).then_inc(osem, 16)
            sync.dma_start(out=out_hz[:, :], in_=obuf_hz[:, 4 : 4 * (T + 1)]).then_inc(osem, 16)
            sync.dma_start(out=out_fb[:, :], in_=obuf_fb[:, :]).then_inc(osem, 16)
            sync.wait_ge(osem, 48)

        @block.gpsimd
        def _(gpsimd):
            gpsimd.memset(one_sb[:, :], 1.0).then_inc(isem, 1)
            gpsimd.memset(hy16_sb[:, :], 0.0).then_inc(isem, 1)
            gpsimd.memset(hyall_sb[:, 32:64], 0.0).then_inc(isem, 1)
            gpsimd.memset(obuf_hy[:, 0:4], 0.0).then_inc(isem, 1)
            gpsimd.memset(obuf_hz[:, 0:4], 0.0).then_inc(isem, 1)
            for s in range(T - 1):
                p = s % 2
                gpsimd.wait_ge(d1sem, 16 * (s + 1))
                if s >= 2:
                    gpsimd.wait_ge(d2sem, 16 * (s - 1))
                gpsimd.collective_compute(
                    "AllGather",
                    mybir.AluOpType.bypass,
                    replica_groups=rg,
                    ins=[agin[p].ap().opt()],
                    outs=[agout[p].ap().opt()],
                ).then_inc(ccsem, 1)

        @block.tensor
        def _(tensor):
            tensor.wait_ge(ldsem, 112)
            tensor.wait_ge(isem, 7)
            for t in range(T):
                p = t % 2
                pm = (t - 1) % 2
                if t >= 2:
                    tensor.wait_ge(predone, t - 1)
                if t >= 1:
                    tensor.wait_ge(ssem, t)
                # psum_rest: W_in@x_t + bias + W_rec@hy_own
                for j in range(4):
                    li = j // 2
                    outp = psum_rest[p][:, j : j + 1]
                    tensor.matmul(
                        outp,
                        win_sb[:, j * 128 : (j + 1) * 128],
                        xt_sb[:, t : t + 1],
                        start=True, stop=False,
                    )
                    tensor.matmul(
                        outp,
                        bias_sb[0:1, j * 128 : (j + 1) * 128],
                        one_sb[0:1, 0:1],
                        start=False, stop=False,
                    )
                    for r in range(2):
                        tensor.matmul(
                            outp,
                            wrec_sb[:, (j * 2 + r) * 128 : (j * 2 + r + 1) * 128],
                            hy16_sb[:, 4 * pm + 2 * li + r : 4 * pm + 2 * li + r + 1],
                            start=False, stop=(r == 1),
                        )
                # psum_fb: folded feedback matvec over gathered hy
                if t >= 1:
                    tensor.wait_ge(d2sem, 16 * t)
                for j in range(4):
                    for cc in range(32):
                        mm = tensor.matmul(
                            psum_fb[p][:, j : j + 1],
                            a_sb[:, (j * 32 + cc) * 128 : (j * 32 + cc + 1) * 128],
                            hyall_sb[:, 32 * pm + cc : 32 * pm + cc + 1],
                            start=(cc == 0), stop=(cc == 31),
                        )
                mm.then_inc(mmsem, 1)
                if t <= T - 2:
                    tensor.wait_ge(castsem, t + 1)
                    tensor.transpose(
                        psum_tr[p][0:4, :],
                        hy16_sb[:, 4 * p : 4 * p + 4],
                        id_sb[:, :],
                    ).then_inc(trsem, 1)

        @block.vector
        def _(vector):
            vector.wait_ge(ldsem, 112)
            for li in range(2):
                vector.tensor_scalar(
                    win_sb[:, li * 256 : (li + 1) * 256],
                    win_sb[:, li * 256 : (li + 1) * 256],
                    mask_sb[:, li : li + 1],
                    None,
                    mybir.AluOpType.mult,
                ).then_inc(isem, 1)
            for t in range(T):
                p = t % 2
                vector.wait_ge(mmsem, t + 1)
                vector.tensor_copy(
                    obuf_fb[:, 4 * t : 4 * t + 4],
                    psum_fb[p][:, :],
                )
                vector.drain()
                vector.tensor_tensor(
                    pre_sb[:, 4 * p : 4 * p + 4],
                    psum_rest[p][:, :],
                    obuf_fb[:, 4 * t : 4 * t + 4],
                    mybir.AluOpType.add,
                ).then_inc(predone, 1)
                vector.wait_ge(tanhsem, t + 1)
                vector.scalar_tensor_tensor(
                    w_sb[:, 4 * p : 4 * p + 4],
                    obuf_hy[:, 4 * t : 4 * t + 4],
                    -GAMMA,
                    tnh_sb[:, 4 * p : 4 * p + 4],
                    mybir.AluOpType.mult,
                    mybir.AluOpType.add,
                )
                vector.wait_ge(zc1sem, t + 1)
                vector.drain()
                vector.scalar_tensor_tensor(
                    obuf_hz[:, 4 * (t + 1) : 4 * (t + 2)],
                    w_sb[:, 4 * p : 4 * p + 4],
                    C3,
                    zc1_sb[:, 4 * p : 4 * p + 4],
                    mybir.AluOpType.mult,
                    mybir.AluOpType.add,
                ).then_inc(zdone, 1)
                vector.drain()
                vector.tensor_tensor(
                    hy16_sb[:, 4 * p : 4 * p + 4],
                    obuf_hy[:, 4 * t : 4 * t + 4],
                    obuf_hz[:, 4 * (t + 1) : 4 * (t + 2)],
                    mybir.AluOpType.add,
                ).then_inc(castsem, 1)
                vector.tensor_tensor(
                    obuf_hy[:, 4 * (t + 1) : 4 * (t + 2)],
                    obuf_hy[:, 4 * t : 4 * t + 4],
                    obuf_hz[:, 4 * (t + 1) : 4 * (t + 2)],
                    mybir.AluOpType.add,
                )
                if t <= T - 2:
                    vector.wait_ge(trsem, t + 1)
                    vector.tensor_copy(
                        tr_sb[0:4, 128 * p : 128 * p + 128],
                        psum_tr[p][0:4, :],
                    ).then_inc(ssem, 1)
                vector.drain()
            # hz output = z / DT
            vector.drain()
            vector.tensor_scalar_mul(
                obuf_hz[:, 4 : 4 * (T + 1)],
                obuf_hz[:, 4 : 4 * (T + 1)],
                1.0 / DT_C,
            ).then_inc(fsem, 1)

        @block.scalar
        def _(scalar):
            scalar.wait_ge(isem, 7)
            for t in range(T):
                p = t % 2
                if t >= 1:
                    scalar.wait_ge(zdone, t)
                scalar.activation(
                    zc1_sb[:, 4 * p : 4 * p + 4],
                    obuf_hz[:, 4 * t : 4 * t + 4],
                    mybir.ActivationFunctionType.Copy,
                    scale=C1,
                ).then_inc(zc1sem, 1)
                scalar.wait_ge(predone, t + 1)
                scalar.activation(
                    tnh_sb[:, 4 * p : 4 * p + 4],
                    pre_sb[:, 4 * p : 4 * p + 4],
                    mybir.ActivationFunctionType.Tanh,
                ).then_inc(tanhsem, 1)

    return nc


def pack_inputs(x, wm, conn, mask, W_in, W_rec, bias):
    """Build the 8 per-core input dicts (host-side layout marshaling)."""
    T = x.shape[0]
    scaling = 1.0 / np.clip(conn.sum(axis=1), 1.0, None)
    abar = (conn[:, :, None, None] * scaling[:, None, None, None] * wm).astype(np.float32)

    xt = np.ascontiguousarray(x.T).astype(BF16_NP)  # [128, T]

    in_maps = []
    for c in range(NCORES):
        a_pack = np.zeros((128, 4 * 32 * 128), np.float32)
        wrec_pack = np.zeros((128, 4 * 2 * 128), np.float32)
        win_pack = np.zeros((128, 4 * 128), np.float32)
        bias_pack = np.zeros((1, 512), np.float32)
        for j in range(4):
            li, h1f = j // 2, j % 2
            i = 2 * c + li
            # A blocks: [p, c32, h1] -> cols (j*32+c32)*128 + mcol
            vals = abar[i][_M_MAT, :, _H_MAT]            # [128, 32, 256]
            vals = vals[:, :, h1f * 128 : (h1f + 1) * 128]  # [128, 32, 128]
            a_pack[:, j * 4096 : (j + 1) * 4096] = vals.reshape(128, 4096)
            wrt = W_rec[i].T                              # [h2, h1]
            for r in range(2):
                wrec_pack[:, (j * 2 + r) * 128 : (j * 2 + r + 1) * 128] = \
                    wrt[r * 128 : (r + 1) * 128, h1f * 128 : (h1f + 1) * 128]
            win_pack[:, j * 128 : (j + 1) * 128] = \
                W_in[i].T[:, h1f * 128 : (h1f + 1) * 128]
            bias_pack[0, j * 128 : (j + 1) * 128] = bias[i, h1f * 128 : (h1f + 1) * 128]
        mask_rep = np.broadcast_to(mask[2 * c : 2 * c + 2][None, :], (128, 2))
        in_maps.append({
            "id_pack": np.eye(128, dtype=BF16_NP),
            "a_pack": a_pack.astype(BF16_NP),
            "wrec_pack": wrec_pack.astype(BF16_NP),
            "win_pack": win_pack.astype(BF16_NP),
            "bias_pack": bias_pack.astype(BF16_NP),
            "xt": xt,
            "mask_rep": np.ascontiguousarray(mask_rep).astype(np.float32),
        })
    return in_maps


def unpack_outputs(results, T):
    states = np.zeros((T, M, 2, H), np.float32)
    fb = np.zeros((T, M, H), np.float32)
    for c in range(NCORES):
        o_hy = results[c]["out_hy"]  # [128, 4T]
        o_hz = results[c]["out_hz"]
        o_fb = results[c]["out_fb"]
        for j in range(4):
            li, h1f = j // 2, j % 2
            m = 2 * c + li
            hs = slice(h1f * 128, (h1f + 1) * 128)
            states[:, m, 0, hs] = o_hy[:, j::4].T
            states[:, m, 1, hs] = o_hz[:, j::4].T
            fb[:, m, hs] = o_fb[:, j::4].T
    return states, fb


_NC_CACHE = {}


def run_packed(in_maps, T, trace=False):
    if T not in _NC_CACHE:
        _NC_CACHE[T] = build_nc(T)
    nc = _NC_CACHE[T]
    res = run_bass_kernel_spmd(nc, in_maps, core_ids=list(range(NCORES)), trace=trace)
    return res


def kernel(x, wm, connection_weights, input_mask, W_in, W_rec, bias):
    x = np.asarray(x, np.float32)
    wm = np.asarray(wm, np.float32)
    conn = np.asarray(connection_weights, np.float32)
    mask = np.asarray(input_mask, np.float32)
    W_in = np.asarray(W_in, np.float32)
    W_rec = np.asarray(W_rec, np.float32)
    bias = np.asarray(bias, np.float32)
    T = x.shape[0]
    in_maps = pack_inputs(x, wm, conn, mask, W_in, W_rec, bias)
    res = run_packed(in_maps, T, trace=False)
    return unpack_outputs(res.results, T)
